# revision 31
# baseline (speedup 1.0000x reference)
"""Trainium2 Bass kernel for nn_Attention (dense_transformer).

Reference computation (per batch b of 4, dim C=256, HEADS=4, hc=64, N=48*48=2304):
  k = wk@x+bk; q = wq@x+bq; v = wv@x+bv          (1x1 convs = channel GEMMs)
  dots[n,m] = sum_c k[c,n] q[c,m]   per head
  attn = softmax(dots, axis=keys n)
  out  = v @ attn ; y = wo@out + bo

Sharding: 8 cores, core c -> (batch c//2, query-half c%2). Each core computes
all 4 heads for its 1152 queries; keys are always the full 2304 positions.
No collectives needed; host reassembles by pure concatenation.

Algebraic folds used on device:
  - bk cancels in softmax over keys (adds a per-query constant to dots).
  - bq folded into q during the PSUM->SBUF copy (per-partition scalar add).
  - bv folded at the end:   y = wo@(att) + (wo@bv + bo)  since sum_n attn = 1.
  - softmax normalizer: ones column appended to v^T so the attn@v matmul
    also produces sumexp; normalization is a per-query reciprocal + broadcast.
"""

import numpy as np

from concourse import bacc, bass, mybir, tile



F32 = mybir.dt.float32
F32R = mybir.dt.float32r
BF16 = mybir.dt.bfloat16

B, C, HW, HEADS, HC = 4, 256, 48, 4, 64
N = HW * HW          # 2304 keys
M = N // 2           # 1152 queries per core
NT = N // 128        # 18 key tiles
WIN = 1536           # exp window (3 PSUM banks)
FLAT = NT * M        # 20736 flat dots cols per head

_CACHED = {}
last_in_maps = None


def _chunks(total, start_align=0):
    """Split [0,total) at multiples of 512 of (start_align + offset)."""
    out = []
    pos = 0
    while pos < total:
        nxt = min(total, ((start_align + pos) // 512 + 1) * 512 - start_align)
        out.append((pos, nxt - pos))
        pos = nxt
    return out


def build_nc():
    nc = bacc.Bacc("TRN2", target_bir_lowering=False, debug=False)

    x_d = nc.dram_tensor("x", [C, N], F32R, kind="ExternalInput")
    xq_d = nc.dram_tensor("xq", [C, M], F32R, kind="ExternalInput")
    wkT_d = nc.dram_tensor("wkT", [C, C], F32R, kind="ExternalInput")
    wqT_d = nc.dram_tensor("wqT", [C, C], F32R, kind="ExternalInput")
    wvT_d = nc.dram_tensor("wvT", [C, C], F32R, kind="ExternalInput")
    woT_d = nc.dram_tensor("woT", [C, C], BF16, kind="ExternalInput")
    bias_d = nc.dram_tensor("bias", [128, 8], F32, kind="ExternalInput")
    y_d = nc.dram_tensor("out", [C, M], F32, kind="ExternalOutput")

    sb = lambda name, shape, dt: nc.alloc_sbuf_tensor(name, shape, dt).ap()

    x_sb = [sb(f"x{i}", [128, N], F32R) for i in range(2)]
    xq_sb = [sb(f"xq{i}", [128, M], F32R) for i in range(2)]
    wkT = [sb(f"wkT{i}", [128, C], F32R) for i in range(2)]
    wqT = [sb(f"wqT{i}", [128, C], F32R) for i in range(2)]
    wvT = [sb(f"wvT{i}", [128, C], F32R) for i in range(2)]
    woT = [sb(f"woT{i}", [128, C], BF16) for i in range(2)]
    bias_sb = sb("bias_sb", [128, 8], F32)
    bqd_sb = bias_sb[:, 0:4]
    bv_sb = bias_sb[:, 4:6]
    bo_sb = bias_sb[:, 6:8]
    # k/q duplicated into both partition halves so dots can row-group-pack
    # (t even -> rows 0:64, t odd -> rows 64:128 run concurrently on PE)
    k2 = [sb(f"k2_{h}", [128, N], BF16) for h in range(HEADS)]
    qb2 = [sb(f"qb2_{h}", [128, M], BF16) for h in range(HEADS)]
    vT_all = sb("vT_all", [128, NT * (HC + 1) * HEADS], BF16)
    VS = (HC + 1) * HEADS
    vT = [vT_all[:, t * VS:(t + 1) * VS] for t in range(NT)]
    # exp output, one tensor per (buffer, m-chunk block) to keep the
    # dots->exp stream free of false WAR deps against attnv reads
    a_blk = [[sb(f"a{i}_{b}", [128, w], BF16) for b, w in
              enumerate([9216, 9216, 2304])] for i in range(2)]
    att = [sb(f"att{i}", [128, M], BF16) for i in range(2)]
    y_sb = [sb(f"y{i}", [128, M], F32) for i in range(2)]
    rcp = sb("rcp", [1, M], F32)
    bcast = sb("bcast", [64, 512], F32)
    fb = sb("fb", [128, 2], F32)

    MC = _chunks(M)  # [(0,512),(512,512),(1024,128)]

    with tile.TileContext(nc) as tc, nc.allow_low_precision(
            reason="attention weights; rel-err budget 2e-2"):
        with (
            tc.tile_pool(name="ps_dots", bufs=2, space="PSUM") as dpool,
            tc.tile_pool(name="ps_acc", bufs=2, space="PSUM") as apool,
        ):
            # ---- input DMAs ----
            # SP ring: bias + k/q weights, then it is free for the k2/qb2
            # duplication DMAs emitted by the projections.  ACT ring: first
            # x piece, xq (query slice), remaining x, then v/o weights.
            nc.sync.dma_start(bias_sb[:, :], bias_d.ap()[:, :])
            for i in range(2):
                cs = slice(i * 128, (i + 1) * 128)
                nc.sync.dma_start(wkT[i][:, :], wkT_d.ap()[cs, :])
                nc.sync.dma_start(wqT[i][:, :], wqT_d.ap()[cs, :])
            for i in range(2):
                cs = slice(i * 128, (i + 1) * 128)
                nc.scalar.dma_start(x_sb[i][:, 0:768], x_d.ap()[cs, 0:768])
            for i in range(2):
                cs = slice(i * 128, (i + 1) * 128)
                nc.scalar.dma_start(xq_sb[i][:, :], xq_d.ap()[cs, :])
            for (c0, cw) in [(768, 768), (1536, 768)]:
                for i in range(2):
                    cs = slice(i * 128, (i + 1) * 128)
                    nc.scalar.dma_start(
                        x_sb[i][:, c0:c0 + cw], x_d.ap()[cs, c0:c0 + cw])
            for i in range(2):
                cs = slice(i * 128, (i + 1) * 128)
                nc.scalar.dma_start(wvT[i][:, :], wvT_d.ap()[cs, :])
                nc.scalar.dma_start(woT[i][:, :], woT_d.ap()[cs, :])
            # warm the ACT exp table while DMAs/projections run
            warm = sb("warm", [1, 2], F32)
            nc.vector.memset(warm[:, :], 0.0)
            nc.scalar.activation(warm[:, :], warm[:, :],
                                 mybir.ActivationFunctionType.Exp)

            # ---- fb = wo@bv + bo (per o-tile column; bf16 matmul) ----
            bv_bf = sb("bv_bf", [128, 2], BF16)
            nc.vector.tensor_copy(bv_bf[:, :], bv_sb[:, :])
            for ot in range(2):
                ps = apool.tile([128, 512], F32, tag="acc")
                for ct in range(2):
                    nc.tensor.matmul(
                        ps[:, 0:1],
                        woT[ct][:, ot * 128:(ot + 1) * 128],
                        bv_bf[:, ct:ct + 1],
                        start=(ct == 0), stop=(ct == 1),
                    )
                nc.vector.tensor_add(fb[:, ot:ot + 1], ps[:, 0:1], bo_sb[:, ot:ot + 1])

            # ---- k/q projections (pair matmuls + per-chunk dup DMAs) ----
            # Each head's k/q is duplicated into both partition halves so
            # dots can row-group-pack by exp-window parity.
            lo, hi = slice(0, 64), slice(64, 128)

            def emit_proj_kq(mt):
                h0, h1 = 2 * mt, 2 * mt + 1
                for (c0, cw) in _chunks(N):
                    ps = apool.tile([128, 512], F32, tag="acc")
                    for ct in range(2):
                        nc.tensor.matmul(
                            ps[:, 0:cw],
                            wkT[ct][:, mt * 128:(mt + 1) * 128],
                            x_sb[ct][:, c0:c0 + cw],
                            start=(ct == 0), stop=(ct == 1),
                        )
                    cs = slice(c0, c0 + cw)
                    nc.vector.tensor_copy(k2[h0][lo, cs], ps[0:64, 0:cw])
                    nc.vector.tensor_copy(k2[h1][hi, cs], ps[64:128, 0:cw])
                    nc.sync.dma_start(k2[h0][hi, cs], k2[h0][lo, cs])
                    nc.sync.dma_start(k2[h1][lo, cs], k2[h1][hi, cs])
                for (c0, cw) in MC:
                    ps = apool.tile([128, 512], F32, tag="acc")
                    for ct in range(2):
                        nc.tensor.matmul(
                            ps[:, 0:cw],
                            wqT[ct][:, mt * 128:(mt + 1) * 128],
                            xq_sb[ct][:, c0:c0 + cw],
                            start=(ct == 0), stop=(ct == 1),
                        )
                    cs = slice(c0, c0 + cw)
                    nc.vector.tensor_scalar_add(
                        qb2[h0][lo, cs], ps[0:64, 0:cw], bqd_sb[0:64, h0:h0 + 1])
                    nc.vector.tensor_scalar_add(
                        qb2[h1][hi, cs], ps[64:128, 0:cw], bqd_sb[64:128, h1:h1 + 1])
                    nc.sync.dma_start(qb2[h0][hi, cs], qb2[h0][lo, cs])
                    nc.sync.dma_start(qb2[h1][lo, cs], qb2[h1][hi, cs])

            emit_proj_kq(0)

            # ---- v^T projection (+ ones column per head for sumexp) ----
            def emit_proj_vT():
                nc.vector.memset(vT_all[:, :], 1.0)
                for t in range(NT):
                    ps = apool.tile([128, 512], F32, tag="acc")
                    for ct in range(2):
                        nc.tensor.matmul(
                            ps[:, 0:C],
                            x_sb[ct][:, t * 128:(t + 1) * 128],
                            wvT[ct][:, :],
                            start=(ct == 0), stop=(ct == 1),
                        )
                    dst = vT[t][:].rearrange("p (h c) -> p h c", c=HC + 1)[:, :, 0:HC]
                    src_ = ps[:, 0:C].rearrange("p (h c) -> p h c", c=HC)
                    nc.vector.tensor_copy(dst, src_)

            # ---- per-head attention, software-pipelined emission ----
            # Flat dots layout is m-chunk-major: col(t, m in chunk b) =
            # BASE[b] + t*MW[b] + (m - M0[b]).  Group Gk(h) = the exp windows
            # of m-chunk k; B(h,k) (attnv+normalize) is ready after Gk(h).
            # Emission order staggers B one group behind the dots stream so
            # the PE always prefers feeding ACT's next exp window.
            BLK = [(0, 512, 0), (512, 512, 9216), (1024, 128, 18432)]
            WIN_OF = [(0, 6), (6, 12), (12, 14)]   # window range per m-chunk
            nwin = (FLAT + WIN - 1) // WIN

            def win_mms(j):
                w0, w1 = j * WIN, min(FLAT, (j + 1) * WIN)
                out = []
                for (m0, mw, base) in BLK:
                    for t in range(NT):
                        c0 = base + t * mw
                        if c0 < w1 and c0 + mw > w0:
                            assert c0 >= w0 and c0 + mw <= w1
                            out.append((t, m0, mw, c0 - w0))
                return out

            def emit_A_group(h, g):
                a = a_blk[h % 2][g]
                base_g = BLK[g][2]
                for j in range(*WIN_OF[g]):
                    w0 = j * WIN
                    wlen = min(WIN, FLAT - w0)
                    D = dpool.tile([128, WIN], F32, tag="dots")
                    rows = slice(0, 64) if j % 2 == 0 else slice(64, 128)
                    for (t, mm0, mmw, doff) in win_mms(j):
                        nc.tensor.matmul(
                            D[:, doff:doff + mmw],
                            k2[h][rows, t * 128:(t + 1) * 128],
                            qb2[h][rows, mm0:mm0 + mmw],
                            start=True, stop=True,
                            tile_position=(rows.start, 0),
                        )
                    nc.scalar.activation(
                        a[:, w0 - base_g:w0 - base_g + wlen], D[:, 0:wlen],
                        mybir.ActivationFunctionType.Exp)

            def emit_B_chunk(h, bi):
                a = a_blk[h % 2][bi]
                m0, mw, base = BLK[bi]
                o2 = apool.tile([128, 512], F32, tag="acc")
                for t in range(NT):
                    nc.tensor.matmul(
                        o2[0:HC + 1, 0:mw],
                        vT[t][:, h * (HC + 1):(h + 1) * (HC + 1)],
                        a[:, t * mw:t * mw + mw],
                        start=(t == 0), stop=(t == NT - 1),
                    )
                nc.vector.reciprocal(rcp[0:1, m0:m0 + mw], o2[HC:HC + 1, 0:mw])
                nc.gpsimd.partition_broadcast(
                    bcast[:, 0:mw], rcp[0:1, m0:m0 + mw])
                nc.vector.tensor_mul(
                    att[h // 2][(h % 2) * 64:(h % 2) * 64 + 64, m0:m0 + mw],
                    o2[0:HC, 0:mw], bcast[:, 0:mw])
                if h == HEADS - 1:
                    emit_unify(bi)

            def emit_unify(bi):
                m0, mw, _ = BLK[bi]
                for ot in range(2):
                    u = apool.tile([128, 512], F32, tag="acc")
                    for ct in range(2):
                        nc.tensor.matmul(
                            u[:, 0:mw],
                            woT[ct][:, ot * 128:(ot + 1) * 128],
                            att[ct][:, m0:m0 + mw],
                            start=(ct == 0), stop=(ct == 1),
                        )
                    nc.vector.tensor_scalar_add(
                        y_sb[ot][:, m0:m0 + mw], u[:, 0:mw], fb[:, ot:ot + 1])
                    nc.sync.dma_start(
                        y_d.ap()[ot * 128:(ot + 1) * 128, m0:m0 + mw],
                        y_sb[ot][:, m0:m0 + mw])

            groups = [(h, g) for h in range(HEADS) for g in range(3)]
            emit_A_group(0, 0)
            emit_proj_kq(1)
            emit_A_group(0, 1)
            emit_proj_vT()
            emit_B_chunk(0, 0)
            for i in range(2, len(groups)):
                emit_A_group(*groups[i])
                emit_B_chunk(*groups[i - 1])
            emit_B_chunk(*groups[-1])

    nc.compile()
    return nc


def _get_nc():
    if "nc" not in _CACHED:
        _CACHED["nc"] = build_nc()
    return _CACHED["nc"]


def kernel(x, wk, bk, wq, bq, wv, bv, wo, bo):
    from concourse import bass_utils

    import ml_dtypes
    bf16 = ml_dtypes.bfloat16
    x = np.ascontiguousarray(np.asarray(x, dtype=np.float32))
    mk = lambda w: np.ascontiguousarray(np.asarray(w, dtype=np.float32).T)
    wkT, wqT, wvT = mk(wk), mk(wq), mk(wv)
    woT = mk(wo).astype(bf16)
    col2 = lambda b: np.asarray(b, dtype=np.float32).reshape(2, 128).T
    bqd = np.asarray(bq, dtype=np.float32).reshape(4, 64)
    bqd = np.concatenate([bqd, bqd], axis=1).T          # [128, 4], both halves
    bias = np.ascontiguousarray(
        np.concatenate([bqd, col2(bv), col2(bo)], axis=1))

    xb = x.reshape(B, C, N)

    nc = _get_nc()
    in_maps = []
    for c in range(8):
        b, qh = c // 2, c % 2
        in_maps.append({
            "x": xb[b],
            "xq": np.ascontiguousarray(xb[b][:, qh * M:(qh + 1) * M]),
            "wkT": wkT, "wqT": wqT, "wvT": wvT, "woT": woT,
            "bias": bias,
        })
    global last_in_maps
    last_in_maps = in_maps
    res = bass_utils.run_bass_kernel_spmd(nc, in_maps, core_ids=list(range(8)))

    out = np.empty((B, C, N), dtype=np.float32)
    for c in range(8):
        b, qh = c // 2, c % 2
        out[b][:, qh * M:(qh + 1) * M] = res.results[c]["out"]
    return out.reshape(B, C, HW, HW)
